# revision 28
# baseline (speedup 1.0000x reference)
"""Trainium2 Bass kernel for nn_GatedJunction (gated multi-branch junction).

Math (per batch element b):
    m_y  = mean_hw(y[b])                     # [C]
    m_xk = mean_hw(x_k[b])                   # [C] for k=0..3
    feats = concat(m_y, m_x0..m_x3)          # [5C] = [1280]
    h  = relu(bn(feats @ conv1_w.T))         # [32]
    w  = h @ conv2_w.T + conv2_b             # [1280] -> [5, 256]
    w1 = sigmoid(w[0])                       # self gate  [256]
    w2 = softmax_k(w[1:])                    # branch gates [4, 256]
    out[b] = y[b]*w1[:,None,None] + sum_k w2[k][:,None,None]*x_k[b]

Sharding: data-parallel over batch. 8 cores x 4 batch elements each.
Params are tiny and replicated to every core.

Layout on-core: channel-on-partition.  Each (tensor, batch) is one SBUF
tile [128, 2, 1024] = [c%128, c//128, h*w].  Channel sums (for the means)
are computed with tensor_scalar(accum_out=...) on DVE (2x fp32 mode) and
activation(accum_out=...) on ACT.  The gate MLP runs on PE with the
contraction chunked by 128 features; conv2 is applied transposed so the
gate logits land directly in channel-on-partition layout [128, 10].
Pass 2 is a chain of per-partition-scalar FMAs (scalar_tensor_tensor)
split across DVE/Pool, with y*w1 on ACT.
"""

import sys

for _p in ("/root/.axon_site/_ro/trn_rl_repo", "/opt/trn_rl_repo"):
    if _p not in sys.path:
        sys.path.append(_p)

from contextlib import ExitStack

import numpy as np

import concourse.bass as bass
import concourse.tile as tile
from concourse import masks, mybir
from concourse.bass_utils import run_bass_kernel_spmd

# Problem constants (hardcoded from the spec).
B, K, C, H, W = 32, 4, 256, 32, 32
MID = 32
EPS = 1e-5
HW = H * W          # 1024
N_CORES = 8
B_LOC = B // N_CORES  # 4
NT = K + 1          # 5 tensors: y, x0..x3
FEAT = NT * C       # 1280
NCH = FEAT // 128   # 10 feature chunks of 128
CH = C // 128       # 2 channel chunks per tensor

FP32 = mybir.dt.float32
ALU = mybir.AluOpType
AF = mybir.ActivationFunctionType


def _split_waits(nc: bass.Bass) -> None:
    """This toolchain's walrus accepts only ONE sync-wait per instruction
    (setupSyncWait: 'Too many sync wait commands') while Tile emits several.
    Hoist all-but-one wait onto standalone EventSemaphore instructions
    placed immediately before, on the same engine — semantically identical
    (sequencer stalls at each wait in order)."""
    for f in nc.m.functions:
        for blk in f.blocks:
            insts = list(blk.instructions)
            out, changed = [], False
            for inst in insts:
                si = inst.sync_info
                if si is not None and len(si.on_wait) > 1:
                    waits = list(si.on_wait)
                    for i, w in enumerate(waits[:-1]):
                        ev = mybir.InstEventSemaphore(
                            name=f"{inst.name}-sw{i}", ins=[], outs=[]
                        )
                        ev.engine = inst.engine
                        ev.sync_info = mybir.SyncInfo(on_wait=[w], on_update=[])
                        out.append(ev)
                    si.on_wait = [waits[-1]]
                    changed = True
                out.append(inst)
            if changed:
                blk.instructions = out


def build_program(debug: bool = False, repeat: int = 1) -> bass.Bass:
    """Emit the single-core SPMD program (same program, per-core data).

    repeat > 1 re-runs the whole batch loop (idempotent) — used only for
    launch-overhead-cancelling timing in test.py.
    """
    nc = bass.Bass()
    if debug:
        d_dbg_mean = nc.declare_dram_parameter("dbg_mean", [B_LOC, 128, NCH], FP32, isOutput=True)
        d_dbg_h = nc.declare_dram_parameter("dbg_h", [B_LOC, MID, 1], FP32, isOutput=True)
        d_dbg_gat = nc.declare_dram_parameter("dbg_gat", [B_LOC, 128, NCH], FP32, isOutput=True)

    d_in = [
        nc.declare_dram_parameter(nm, [B_LOC, CH, 128, HW], FP32, isOutput=False)
        for nm in ("y", "x0", "x1", "x2", "x3")
    ]
    d_c1 = nc.declare_dram_parameter("conv1_w", [MID, FEAT], FP32, isOutput=False)
    d_gamma = nc.declare_dram_parameter("bn_gamma", [MID, 1], FP32, isOutput=False)
    d_beta = nc.declare_dram_parameter("bn_beta", [MID, 1], FP32, isOutput=False)
    d_mean = nc.declare_dram_parameter("bn_mean", [MID, 1], FP32, isOutput=False)
    d_var = nc.declare_dram_parameter("bn_var", [MID, 1], FP32, isOutput=False)
    d_c2 = nc.declare_dram_parameter("conv2_w", [NCH, 128, MID], FP32, isOutput=False)
    d_c2b = nc.declare_dram_parameter("conv2_b", [NCH, 128], FP32, isOutput=False)
    d_out = nc.declare_dram_parameter("out", [B_LOC, CH, 128, HW], FP32, isOutput=True)

    with tile.TileContext(nc) as tc, ExitStack() as ctx:
        cpool = ctx.enter_context(tc.tile_pool(name="cpool", bufs=1))
        ppool = ctx.enter_context(tc.tile_pool(name="ppool", bufs=2, space="PSUM"))
        dpool = ctx.enter_context(tc.tile_pool(name="dpool", bufs=2))
        spool = ctx.enter_context(tc.tile_pool(name="spool", bufs=2))

        # ---------------- parameter prep (once) ----------------
        # Transposed param layouts via DMA-transpose straight from DRAM, then
        # "laundered" through one DVE copy each so PE matmuls (which tolerate
        # only ONE sync-wait on their embedded fp32 weight load) depend on a
        # single producer proc (DVE).
        # conv1_w [32, 1280] -> w1T [128, (j, m)]  (w1T[p, j, m] = conv1_w[m, 128j+p])
        w1s = cpool.tile([128, NCH, MID], FP32, name="w1s", tag="w1s")
        w1T = cpool.tile([128, NCH, MID], FP32, name="w1T", tag="w1T")
        for j in range(NCH):
            nc.sync.dma_start(
                out=w1s[:, j, :],
                in_=d_c1[:, j * 128 : (j + 1) * 128].rearrange("m p -> p m"),
            )
            nc.vector.tensor_copy(w1T[:, j, :], w1s[:, j, :])

        # conv2_w [(j p), m] -> w2T [32, (j, p)]  (w2T[m, j, p] = conv2_w[128j+p, m])
        w2s = cpool.tile([MID, NCH, 128], FP32, name="w2s", tag="w2s")
        w2T = cpool.tile([MID, NCH, 128], FP32, name="w2T", tag="w2T")
        for j in range(NCH):
            nc.sync.dma_start(out=w2s[:, j, :], in_=d_c2[j].rearrange("p m -> m p"))
            nc.vector.tensor_copy(w2T[:, j, :], w2s[:, j, :])

        # conv2_b [j, p] -> c2bT [128, j]
        c2bs = cpool.tile([128, NCH], FP32, name="c2bs", tag="c2bs")
        nc.sync.dma_start(out=c2bs[:], in_=d_c2b.rearrange("j p -> p j"))
        c2bT = cpool.tile([128, NCH], FP32, name="c2bT", tag="c2bT")
        nc.vector.tensor_copy(c2bT[:], c2bs[:])

        # BN folded affine: h_bn = h_raw * scale_eff + bias_eff, where
        # h_raw = conv1_w @ sums (sums = means * HW), s = gamma/sqrt(var+eps),
        # scale_eff = s / HW, bias_eff = beta - mean * s.
        bn_g = cpool.tile([MID, 1], FP32, name="bn_g", tag="bn_g")
        bn_b = cpool.tile([MID, 1], FP32, name="bn_b", tag="bn_b")
        bn_m = cpool.tile([MID, 1], FP32, name="bn_m", tag="bn_m")
        bn_v = cpool.tile([MID, 1], FP32, name="bn_v", tag="bn_v")
        nc.sync.dma_start(out=bn_g[:], in_=d_gamma[:])
        nc.sync.dma_start(out=bn_b[:], in_=d_beta[:])
        nc.sync.dma_start(out=bn_m[:], in_=d_mean[:])
        nc.sync.dma_start(out=bn_v[:], in_=d_var[:])
        veps = cpool.tile([MID, 1], FP32, name="veps", tag="veps")
        nc.vector.tensor_scalar_add(out=veps[:], in0=bn_v[:], scalar1=float(EPS))
        sq = cpool.tile([MID, 1], FP32, name="sq", tag="sq")
        nc.scalar.sqrt(out=sq[:], in_=veps[:])
        inv = cpool.tile([MID, 1], FP32, name="inv", tag="inv")
        nc.vector.reciprocal(inv[:], sq[:])
        s_bn = cpool.tile([MID, 1], FP32, name="s_bn", tag="s_bn")
        nc.vector.tensor_tensor(out=s_bn[:], in0=bn_g[:], in1=inv[:], op=ALU.mult)
        scale_eff = cpool.tile([MID, 1], FP32, name="scale_eff", tag="scale_eff")
        nc.vector.tensor_scalar_mul(out=scale_eff[:], in0=s_bn[:], scalar1=1.0 / HW)
        ms = cpool.tile([MID, 1], FP32, name="ms", tag="ms")
        nc.vector.tensor_tensor(out=ms[:], in0=bn_m[:], in1=s_bn[:], op=ALU.mult)
        bias_eff = cpool.tile([MID, 1], FP32, name="bias_eff", tag="bias_eff")
        nc.vector.tensor_tensor(out=bias_eff[:], in0=bn_b[:], in1=ms[:], op=ALU.subtract)

        # ---------------- main loop over local batches ----------------
        for b in [i % B_LOC for i in range(B_LOC * repeat)]:
            # Load the 5 feature maps for this batch: [128, ch, hw].
            tiles = []
            for t in range(NT):
                dt_ = dpool.tile(
                    [128, CH, HW], FP32, name=f"d{t}", tag=f"d{t}", bufs=2
                )
                nc.sync.dma_start(out=dt_[:], in_=d_in[t][b].rearrange("c p f -> p c f"))
                tiles.append(dt_)

            # Channel sums -> mean_t[:, j], j = t*CH + ch.
            # All on DVE (tensor_scalar accum, 2x fp32) so PE's matmul waits
            # collapse to a single proc condition (HW sync-wait limit).
            mean_t = spool.tile([128, NCH], FP32, name="mean_t", tag="mean_t", bufs=2)
            for ch in range(CH):  # y sums on ACT (keeps DVE under the DMA bound)
                scr_a = spool.tile([128, HW], FP32, name="scr_a", tag="scr_a", bufs=2)
                nc.scalar.activation(
                    out=scr_a[:],
                    in_=tiles[0][:, ch, :],
                    func=AF.Copy,
                    accum_out=mean_t[:, ch : ch + 1],
                )
            for t in range(1, NT):
                for ch in range(CH):
                    j = t * CH + ch
                    scr_v = spool.tile(
                        [128, HW], FP32, name="scr_v", tag="scr_v", bufs=2
                    )
                    nc.vector.tensor_scalar(
                        out=scr_v[:],
                        in0=tiles[t][:, ch, :],
                        scalar1=1.0,
                        scalar2=None,
                        op0=ALU.mult,
                        op1=ALU.add,
                        accum_out=mean_t[:, j : j + 1],
                    )

            # Gate MLP on PE: h_raw[mid] = sum_j w1T[:,j,:].T @ sums[:,j]
            hps = ppool.tile([MID, 1], FP32, name="hps", tag="hps")
            for j in range(NCH):
                nc.tensor.matmul(
                    hps[:],
                    w1T[:, j, :],
                    mean_t[:, j : j + 1],
                    start=(j == 0),
                    stop=(j == NCH - 1),
                )
            h_sb = spool.tile([MID, 1], FP32, name="h_sb", tag="h_sb", bufs=2)
            nc.scalar.activation(
                out=h_sb[:], in_=hps[:], func=AF.Relu,
                bias=bias_eff[:], scale=scale_eff[:],
            )
            # Logits (pre-bias), transposed into channel-on-partition layout:
            # wps[p, j] = w[128j + p] - conv2_b[128j + p]
            wps = ppool.tile([128, NCH], FP32, name="wps", tag="wps")
            for j in range(NCH):
                nc.tensor.matmul(
                    wps[:, j : j + 1], w2T[:, j, :], h_sb[:], start=True, stop=True
                )

            # Gates: cols 0..1 = sigmoid self gate; cols 2..9 = exp for softmax.
            gat = spool.tile([128, NCH], FP32, name="gat", tag="gat", bufs=2)
            for ch in range(CH):
                nc.scalar.activation(
                    out=gat[:, ch : ch + 1], in_=wps[:, ch : ch + 1],
                    func=AF.Sigmoid, bias=c2bT[:, ch : ch + 1],
                )
            for j in range(CH, NCH):
                nc.scalar.activation(
                    out=gat[:, j : j + 1], in_=wps[:, j : j + 1],
                    func=AF.Exp, bias=c2bT[:, j : j + 1],
                )
            # softmax over k: columns 2+2k+ch, k=0..3.
            gk = gat[:, CH:NCH].rearrange("p (k c) -> p c k", c=CH)
            esum = spool.tile([128, CH, 1], FP32, name="esum", tag="esum", bufs=2)
            nc.vector.reduce_sum(out=esum[:], in_=gk, axis=mybir.AxisListType.X)
            rinv = spool.tile([128, CH, 1], FP32, name="rinv", tag="rinv", bufs=2)
            nc.vector.reciprocal(rinv[:], esum[:])
            for ch in range(CH):
                nc.vector.tensor_scalar_mul(
                    out=gk[:, ch, :], in0=gk[:, ch, :], scalar1=rinv[:, ch, :]
                )

            if debug:
                nc.sync.dma_start(out=d_dbg_mean[b], in_=mean_t[:])
                nc.sync.dma_start(out=d_dbg_h[b], in_=h_sb[:])
                nc.sync.dma_start(out=d_dbg_gat[b], in_=gat[:])

            # Pass 2: acc = y*w1 + sum_k x_k * g_k, then store.
            acc = dpool.tile([128, CH, HW], FP32, name="acc", tag="acc", bufs=2)
            for ch in range(CH):
                nc.scalar.activation(
                    out=acc[:, ch, :], in_=tiles[0][:, ch, :],
                    func=AF.Copy, scale=gat[:, ch : ch + 1],
                )
                for k in range(K):
                    nc.vector.scalar_tensor_tensor(
                        out=acc[:, ch, :],
                        in0=tiles[1 + k][:, ch, :],
                        scalar=gat[:, CH + CH * k + ch : CH + CH * k + ch + 1],
                        in1=acc[:, ch, :],
                        op0=ALU.mult,
                        op1=ALU.add,
                    )
            nc.sync.dma_start(out=d_out[b].rearrange("c p f -> p c f"), in_=acc[:])

    _split_waits(nc)
    return nc


_CACHE: dict = {}


def _get_program() -> bass.Bass:
    if "nc" not in _CACHE:
        _CACHE["nc"] = build_program()
    return _CACHE["nc"]


def make_in_maps(inputs: dict) -> list:
    """Shard full inputs into per-core input maps (batch-parallel)."""
    f32 = lambda a: np.ascontiguousarray(np.asarray(a), dtype=np.float32)
    y = f32(inputs["y"]).reshape(B, CH, 128, HW)
    xs = [f32(inputs[f"x{k}"]).reshape(B, CH, 128, HW) for k in range(K)]
    shared = {
        "conv1_w": f32(inputs["conv1_w"]),
        "bn_gamma": f32(inputs["bn_gamma"]).reshape(MID, 1),
        "bn_beta": f32(inputs["bn_beta"]).reshape(MID, 1),
        "bn_mean": f32(inputs["bn_mean"]).reshape(MID, 1),
        "bn_var": f32(inputs["bn_var"]).reshape(MID, 1),
        "conv2_w": f32(inputs["conv2_w"]).reshape(NCH, 128, MID),
        "conv2_b": f32(inputs["conv2_b"]).reshape(NCH, 128),
    }
    in_maps = []
    for core in range(N_CORES):
        sl = slice(core * B_LOC, (core + 1) * B_LOC)
        m = {"y": np.ascontiguousarray(y[sl])}
        for k in range(K):
            m[f"x{k}"] = np.ascontiguousarray(xs[k][sl])
        m.update(shared)
        in_maps.append(m)
    return in_maps


def kernel(**inputs) -> np.ndarray:
    nc = _get_program()
    in_maps = make_in_maps(inputs)
    res = run_bass_kernel_spmd(nc, in_maps, list(range(N_CORES)))
    _CACHE["last_results"] = res
    out = np.concatenate(
        [res.results[i]["out"].reshape(B_LOC, C, H, W) for i in range(N_CORES)],
        axis=0,
    )
    return out.astype(np.float32)
